# revision 26
# baseline (speedup 1.0000x reference)
"""Single-head cross-attention (layernorm + QKV proj + softmax(QK^T)V) on 8 NeuronCores.

Sharding: data-parallel over batch B=8, one batch element per core.

Per-core device program (all matmuls bf16 with fp32 PSUM accumulation), structured
to keep the PE array streaming back-to-back:

  - Host pre-casts x to bf16 and pre-transposes/folds weights; biases and the
    broadcast v-bias are packed into one constants tensor (single DMA).
  - All projection weights live resident in SBUF (one contiguous DMA each).
  - Tile serializes every DMA-xbar transpose against ALL in-flight DMA copies
    (HW deadlock guard), so transposes are batched: LN runs in-place on half-
    phase batches (8 token tiles), then 8 back-to-back transposes flip the
    batch to [d, token] layout. Only ~4 xbar transitions per phase.
  - Projections produce qT/kT in [e, token] layout and v in natural [token, e]
    layout, so attention needs no further transposes.
  - scores^T[j, i] = kT^T qT, exp via ScalarE (scale 1/sqrt(d) fused, no max
    subtraction: |scores*scale| < 3), giving unnormalized attn^T in bf16.
  - out_u[i, e] = attn^T.T @ v and Z[i] = attn^T.T @ ones accumulate in PSUM;
    out = out_u / Z. The attention loop is software-pipelined one chunk deep
    (issue order S0, S1, A0, S2, A1, ...) so attv never waits on exp.
"""

import os
from contextlib import ExitStack

import numpy as np
import ml_dtypes

import concourse.bass as bass
import concourse.bacc as bacc
import concourse.mybir as mybir
import concourse.tile as tile
from concourse.bass import ts, ds
from concourse.tile_rust import add_dep_helper
from concourse.bass_utils import run_bass_kernel_spmd

BF16 = mybir.dt.bfloat16
F32 = mybir.dt.float32
F8E4 = mybir.dt.float8e4

B, T, D = 8, 2048, 1024
EPS = 1e-5
SCALE = float(D) ** -0.5
P = 128
N_IT = T // P          # 16 token tiles of 128
N_DB = D // P          # 8 d-blocks of 128
N_EB = D // P          # 8 e-blocks of 128
N_IC = T // 512        # 4 token chunks of 512
N_EC = D // 512        # 2 e chunks of 512
CT = 4                 # token tiles per chunk


def build_module() -> bass.Bass:
    nc = bacc.Bacc("TRN2", target_bir_lowering=False)

    x_t = nc.dram_tensor("x_t", [T, D], BF16, kind="ExternalInput")
    x_k = nc.dram_tensor("x_k", [T, D], BF16, kind="ExternalInput")
    x_v = nc.dram_tensor("x_v", [T, D], BF16, kind="ExternalInput")
    wq = nc.dram_tensor("wq", [D, D], BF16, kind="ExternalInput")  # pre-transposed [d, e]
    wk = nc.dram_tensor("wk", [D, D], BF16, kind="ExternalInput")
    wv = nc.dram_tensor("wv", [D, D], BF16, kind="ExternalInput")
    # consts[p, 0:8]=bq  [p, 8:16]=bk  [p, 16:1040]=bv broadcast
    consts = nc.dram_tensor("consts", [P, 16 + D], F32, kind="ExternalInput")
    out = nc.dram_tensor("out", [T, D], F32, kind="ExternalOutput")

    with tile.TileContext(nc) as tc, ExitStack() as ctx:
        const = ctx.enter_context(tc.tile_pool(name="const", bufs=1))
        qkv = ctx.enter_context(tc.tile_pool(name="qkv", bufs=1))

        eps_t = const.tile([P, 1], F32)
        nc.vector.memset(eps_t, EPS)
        ones_t = const.tile([P, 1], BF16)
        nc.vector.memset(ones_t, 1.0)
        c_sb = const.tile([P, 16 + D], F32)
        bq_sb = c_sb[:, 0:N_EB]
        bk_sb = c_sb[:, N_EB:2 * N_EB]
        bv_bc = c_sb[:, 16:16 + D]

        # ---- persistent projection outputs ----
        # qT/kT in fp8e4 (e4m3): feeds the DoubleRow scores matmul at 2x rate.
        qT = qkv.tile([P, N_EB, T], F8E4)        # qT[p, eb, i] = q[i, eb*128+p]
        kT = qkv.tile([P, N_EB, T], F8E4)
        v_sb = qkv.tile([P, N_IT, D], BF16)      # v[p, jt, e] = v[jt*128+p, e]

        with tc.tile_pool(name="proj_phase", bufs=1) as pp, \
             tc.tile_pool(name="proj_ps", bufs=8, space="PSUM") as proj_ps:
            w_sbs = {}
            tp_hist = []  # last xbar-transpose instruction of each batch

            def order_after_transposes(dma_inst, lag=2):
                # Schedule copies after a past transpose batch: the scheduler
                # otherwise slots them inside a batch, and every transpose
                # serializes against in-flight copies (xbar guard). lag=2 lets
                # loads land in the gap between batches a window early; lag=1
                # keeps bulky weight loads strictly behind the current batch.
                tgt = tp_hist[-lag] if len(tp_hist) >= lag else \
                    (tp_hist[-1] if tp_hist else None)
                if tgt is not None:
                    add_dep_helper(dma_inst.ins, tgt.ins, False,
                                   reason="copy after transpose batch (xbar)")

            def load_weight(name, w_dram):
                # w_sb[p, a, e] = w[a*128+p, e]; 2KB contiguous descriptor rows
                w_sbs[name] = pp.tile([P, N_DB, D], BF16, tag=f"w_{name}", bufs=1,
                                      name=f"w_sb_{name}")
                wi = nc.gpsimd.dma_start(out=w_sbs[name],
                                         in_=w_dram[:, :].rearrange("(a p) e -> p a e", p=P))
                order_after_transposes(wi, lag=1)

            x_chunks = {}

            def load_chunk(pname, x_dram, ic):
                xc = pp.tile([P, CT, D], BF16, tag="x_chunk", bufs=4,
                             name=f"x_{pname}_{ic}")
                li = nc.gpsimd.dma_start(
                    out=xc,
                    in_=x_dram[ts(ic, 512), :].rearrange("(s p) d -> p s d", p=P),
                )
                order_after_transposes(li, lag=1)
                x_chunks[(pname, ic)] = xc

            def ln_chunk(pname, ic):
                """In-place layernorm of one loaded chunk; returns the tile."""
                xc = x_chunks.pop((pname, ic))
                held = []
                for s in range(CT):
                    stats = pp.tile([P, 2, 6], F32, tag="stats", bufs=8,
                                    name=f"stats_{pname}_{ic}_{s}")
                    for sb in range(2):
                        nc.vector.bn_stats(out=stats[:, sb, :], in_=xc[:, s, ts(sb, 512)])
                    mv = pp.tile([P, 2], F32, tag="mv", bufs=8,
                                 name=f"mv_{pname}_{ic}_{s}")
                    nc.vector.bn_aggr(out=mv, in_=stats)
                    rstd = pp.tile([P, 1], F32, tag="rstd", bufs=8,
                                   name=f"rstd_{pname}_{ic}_{s}")
                    nc.scalar.activation(
                        out=rstd, in_=mv[:, 1:2],
                        func=mybir.ActivationFunctionType.Abs_reciprocal_sqrt,
                        bias=eps_t,
                    )
                    held.append((s, mv, rstd))
                for s, mv, rstd in held:
                    nc.vector.tensor_scalar(
                        out=xc[:, s, :], in0=xc[:, s, :], scalar1=mv[:, 0:1],
                        scalar2=rstd,
                        op0=mybir.AluOpType.subtract, op1=mybir.AluOpType.mult,
                    )
                return xc

            def transpose_half(xcs, xt_half):
                """8 back-to-back xbar transposes: one batch, few mode flips.

                xt_half[p, db, u*128+q] = ln[u*128+q (token), db*128+p].
                """
                last = None
                for u, (xc, s) in enumerate(xcs):
                    last = nc.sync.dma_start(
                        out=xt_half[:, :, ts(u, P)], in_=xc[:, s, :], transpose=True)
                tp_hist.append(last)

            def proj_chunkT(w_sb, bias_sb, xt_half, off, dstT, pname, ic):
                """dstT[:, eb, ic*512:+512] = w^T @ ln_chunk + bias."""
                for eb in range(N_EB):
                    ps = proj_ps.tile([P, 512], F32, tag="mm", name=f"ps_{pname}_{eb}_{ic}")
                    for db in range(N_DB):
                        nc.tensor.matmul(
                            ps, lhsT=w_sb[:, db, ts(eb, P)],
                            rhs=xt_half[:, db, ds(off, 512)],
                            start=(db == 0), stop=(db == N_DB - 1),
                        )
                    nc.scalar.activation(
                        out=dstT[:, eb, ts(ic, 512)], in_=ps,
                        func=mybir.ActivationFunctionType.Identity,
                        bias=bias_sb[:, eb:eb + 1],
                    )

            def proj_chunkV(w_sb, xt_half, off, ic):
                """v_sb[:, ic*4+s, :] = ln_chunk^T @ w + bias (natural layout)."""
                for s in range(CT):
                    jt = ic * CT + s
                    for ec in range(N_EC):
                        ps = proj_ps.tile([P, 512], F32, tag="mm",
                                        name=f"ps_v_{jt}_{ec}")
                        for db in range(N_DB):
                            nc.tensor.matmul(
                                ps,
                                lhsT=xt_half[:, db, ds(off + s * P, P)],
                                rhs=w_sb[:, db, ts(ec, 512)],
                                start=(db == 0), stop=(db == N_DB - 1),
                            )
                        nc.vector.tensor_add(
                            out=v_sb[:, jt, ts(ec, 512)], in0=ps,
                            in1=bv_bc[:, ts(ec, 512)],
                        )

            # ---- schedule ----------------------------------------------------
            # Meta-batches interleave the independent t and k phases, and the
            # pending-proj deque is TWO deep: every LN+transpose chain gets two
            # PE windows of slack before its projection needs the data.
            proj_fns = {
                "t": lambda xt, off, ic: proj_chunkT(w_sbs["q"], bq_sb, xt, off, qT, "q", ic),
                "k": lambda xt, off, ic: proj_chunkT(w_sbs["k"], bk_sb, xt, off, kT, "k", ic),
                "v": lambda xt, off, ic: proj_chunkV(w_sbs["v"], xt, off, ic),
            }
            x_drams = {"t": x_t, "k": x_k, "v": x_v}
            metas = [("t", [0]), ("k", [0]), ("t", [1]), ("k", [1]),
                     ("t", [2, 3]), ("k", [2, 3]), ("v", [0, 1]), ("v", [2, 3])]

            # global load order = LN consumption order, prefetched ahead
            order = [(pn, ic) for pn, batch in metas for ic in batch]
            n_loaded = [0]

            def ensure_loaded(upto):
                while n_loaded[0] <= min(upto, len(order) - 1):
                    pn, ic = order[n_loaded[0]]
                    load_chunk(pn, x_drams[pn], ic)
                    n_loaded[0] += 1

            ensure_loaded(0)
            load_weight("q", wq)
            nc.gpsimd.dma_start(out=c_sb, in_=consts[:, :])
            # wk/wv issued after the first transpose batches: in-flight copies
            # block transposes (xbar guard), so keep the head's DMA queue lean
            post_batch = {0: lambda: load_weight("k", wk),
                          2: lambda: load_weight("v", wv)}

            g = 0          # chunks fully issued through LN
            pend = []      # [(proj_fn, xt_half, [(off, ic), ...])], 2 deep
            for mi, (pname, batch) in enumerate(metas):
                with nc.named_scope(f"b_{pname}{batch[0]}"):
                    if len(pend) >= 2:
                        pfn, pxt, pchunks = pend.pop(0)
                        for off, ic in pchunks:
                            pfn(pxt, off, ic)
                    xcs = []
                    for ic in batch:
                        xc = ln_chunk(pname, ic)
                        xcs.extend((xc, s) for s in range(CT))
                        g += 1
                    xt_half = pp.tile([P, N_DB, len(batch) * 512], BF16,
                                      tag="xt", bufs=3,
                                      name=f"xt_{pname}_{batch[0]}")
                    transpose_half(xcs, xt_half)
                    # prefetch loads only BETWEEN transpose batches: a load
                    # scheduled among the transposes splits the batch (each
                    # transpose xbar-serializes against prior copies)
                    ensure_loaded(g + 2)
                    if mi in post_batch:
                        post_batch[mi]()
                    pend.append((proj_fns[pname], xt_half,
                                 [(j * 512, ic) for j, ic in enumerate(batch)]))
            for pfn, pxt, pchunks in pend:
                for off, ic in pchunks:
                    pfn(pxt, off, ic)

        # ---- attention, software-pipelined one chunk deep ----
        with tc.tile_pool(name="attv_ps", bufs=2, space="PSUM") as attv_ps, \
             tc.tile_pool(name="mm_ps", bufs=4, space="PSUM") as mm_ps, \
             tc.tile_pool(name="att", bufs=1) as att:
            aTs = {}

            def scores_chunk(ic):
                aT = att.tile([P, N_IT, 512], BF16, tag="aT", bufs=2, name=f"aT_{ic}")
                aTs[ic] = aT
                with nc.named_scope(f"scores_{ic}"):
                    for jt in range(N_IT):
                        ps = mm_ps.tile([P, 512], F32, tag="mm", name=f"ps_s_{ic}_{jt}")
                        # fp8e4 DoubleRow: contraction over eb pairs at 2x rate
                        for a in range(N_EB // 2):
                            nc.tensor.matmul(
                                ps, lhsT=kT[:, ds(2 * a, 2), ts(jt, P)],
                                rhs=qT[:, ds(2 * a, 2), ts(ic, 512)],
                                start=(a == 0), stop=(a == N_EB // 2 - 1),
                                perf_mode=mybir.MatmulPerfMode.DoubleRow,
                            )
                        nc.scalar.activation(
                            out=aT[:, jt, :], in_=ps,
                            func=mybir.ActivationFunctionType.Exp, scale=SCALE,
                        )

            def attv_chunk(ic):
                aT = aTs.pop(ic)
                with nc.named_scope(f"attv_{ic}"):
                    for isub in range(4):
                        ou = attv_ps.tile([P, D], F32, tag="ou", name=f"ou_{ic}_{isub}")
                        zz = mm_ps.tile([P, 1], F32, tag="mm", name=f"z_{ic}_{isub}")
                        # same-bank runs of 16 accumulating matmuls (bank cycling
                        # between consecutive matmuls forces PE micro-stalls)
                        for ec in range(N_EC):
                            for jt in range(N_IT):
                                nc.tensor.matmul(
                                    ou[:, ts(ec, 512)], lhsT=aT[:, jt, ts(isub, P)],
                                    rhs=v_sb[:, jt, ts(ec, 512)],
                                    start=(jt == 0), stop=(jt == N_IT - 1))
                        for jt in range(N_IT):
                            nc.tensor.matmul(zz, lhsT=aT[:, jt, ts(isub, P)], rhs=ones_t,
                                             start=(jt == 0), stop=(jt == N_IT - 1))
                        rz = att.tile([P, 1], F32, tag="rz", bufs=2,
                                      name=f"rz_{ic}_{isub}")
                        nc.vector.reciprocal(out=rz, in_=zz)
                        o_sb = att.tile([P, D], F32, tag="o_sb", bufs=2,
                                        name=f"o_{ic}_{isub}")
                        nc.vector.tensor_scalar_mul(out=o_sb, in0=ou, scalar1=rz)
                        nc.sync.dma_start(out=out[ts(ic * 4 + isub, P), :], in_=o_sb)

            scores_chunk(0)
            for ic in range(N_IC):
                if ic + 1 < N_IC:
                    scores_chunk(ic + 1)
                attv_chunk(ic)

    nc.compile()
    return nc


_NC_CACHE = None


def _get_module():
    global _NC_CACHE
    if _NC_CACHE is None:
        _NC_CACHE = build_module()
    return _NC_CACHE


def kernel(target, source_k, source_v, Wq, bq, Wk, bk, Wv, bv,
           g_t, b_t, g_k, b_k, g_v, b_v):
    target = np.asarray(target, dtype=np.float32)
    source_k = np.asarray(source_k, dtype=np.float32)
    source_v = np.asarray(source_v, dtype=np.float32)
    Wq = np.asarray(Wq, dtype=np.float32); bq = np.asarray(bq, dtype=np.float32)
    Wk = np.asarray(Wk, dtype=np.float32); bk = np.asarray(bk, dtype=np.float32)
    Wv = np.asarray(Wv, dtype=np.float32); bv = np.asarray(bv, dtype=np.float32)
    g_t = np.asarray(g_t, dtype=np.float32); b_t = np.asarray(b_t, dtype=np.float32)
    g_k = np.asarray(g_k, dtype=np.float32); b_k = np.asarray(b_k, dtype=np.float32)
    g_v = np.asarray(g_v, dtype=np.float32); b_v = np.asarray(b_v, dtype=np.float32)

    bf16 = ml_dtypes.bfloat16
    # Fold the layernorm affine (g, b) into the projection weights/biases:
    #   LN_affine(x) @ W.T + b  ==  LN_plain(x) @ (W*g).T + (b + W @ b_ln)
    wqT = np.ascontiguousarray((Wq * g_t[None, :]).T).astype(bf16)
    wkT = np.ascontiguousarray((Wk * g_k[None, :]).T).astype(bf16)
    wvT = np.ascontiguousarray((Wv * g_v[None, :]).T).astype(bf16)
    bq_f = bq + Wq @ b_t
    bk_f = bk + Wk @ b_k
    bv_f = bv + Wv @ b_v

    # consts[p, 0:8]=bq[a*128+p], [p, 8:16]=bk[a*128+p], [p, 16:]=bv broadcast
    consts = np.empty((P, 16 + D), np.float32)
    consts[:, 0:8] = bq_f.reshape(8, P).T
    consts[:, 8:16] = bk_f.reshape(8, P).T
    consts[:, 16:] = bv_f[None, :]

    x_t8 = target.astype(bf16)
    x_k8 = source_k.astype(bf16)
    x_v8 = source_v.astype(bf16)

    nc = _get_module()
    in_maps = []
    for b in range(B):
        in_maps.append({
            "x_t": np.ascontiguousarray(x_t8[b]),
            "x_k": np.ascontiguousarray(x_k8[b]),
            "x_v": np.ascontiguousarray(x_v8[b]),
            "wq": wqT, "wk": wkT, "wv": wvT,
            "consts": consts,
        })

    res = run_bass_kernel_spmd(nc, in_maps, core_ids=list(range(B)),
                               trace=bool(int(os.environ.get("KERNEL_TRACE", "0"))))
    out = np.stack([res.results[b]["out"] for b in range(B)], axis=0)
    kernel.last_results = res
    return out


# revision 27
# speedup vs baseline: 1.0293x; 1.0293x over previous
"""Single-head cross-attention (layernorm + QKV proj + softmax(QK^T)V) on 8 NeuronCores.

Sharding: data-parallel over batch B=8, one batch element per core.

Per-core device program (all matmuls bf16 with fp32 PSUM accumulation), structured
to keep the PE array streaming back-to-back:

  - Host pre-casts x to bf16 and pre-transposes/folds weights; biases and the
    broadcast v-bias are packed into one constants tensor (single DMA).
  - All projection weights live resident in SBUF (one contiguous DMA each).
  - Tile serializes every DMA-xbar transpose against ALL in-flight DMA copies
    (HW deadlock guard), so transposes are batched: LN runs in-place on half-
    phase batches (8 token tiles), then 8 back-to-back transposes flip the
    batch to [d, token] layout. Only ~4 xbar transitions per phase.
  - Projections produce qT/kT in [e, token] layout and v in natural [token, e]
    layout, so attention needs no further transposes.
  - scores^T[j, i] = kT^T qT, exp via ScalarE (scale 1/sqrt(d) fused, no max
    subtraction: |scores*scale| < 3), giving unnormalized attn^T in bf16.
  - out_u[i, e] = attn^T.T @ v and Z[i] = attn^T.T @ ones accumulate in PSUM;
    out = out_u / Z. The attention loop is software-pipelined one chunk deep
    (issue order S0, S1, A0, S2, A1, ...) so attv never waits on exp.
"""

import os
from contextlib import ExitStack

import numpy as np
import ml_dtypes

import concourse.bass as bass
import concourse.bacc as bacc
import concourse.mybir as mybir
import concourse.tile as tile
from concourse.bass import ts, ds
from concourse.tile_rust import add_dep_helper
from concourse.bass_utils import run_bass_kernel_spmd

BF16 = mybir.dt.bfloat16
F32 = mybir.dt.float32
F8E4 = mybir.dt.float8e4

B, T, D = 8, 2048, 1024
EPS = 1e-5
SCALE = float(D) ** -0.5
P = 128
N_IT = T // P          # 16 token tiles of 128
N_DB = D // P          # 8 d-blocks of 128
N_EB = D // P          # 8 e-blocks of 128
N_IC = T // 512        # 4 token chunks of 512
N_EC = D // 512        # 2 e chunks of 512
CT = 4                 # token tiles per chunk


def build_module() -> bass.Bass:
    nc = bacc.Bacc("TRN2", target_bir_lowering=False)

    x_t = nc.dram_tensor("x_t", [T, D], BF16, kind="ExternalInput")
    x_k = nc.dram_tensor("x_k", [T, D], BF16, kind="ExternalInput")
    x_v = nc.dram_tensor("x_v", [T, D], BF16, kind="ExternalInput")
    wq = nc.dram_tensor("wq", [D, D], BF16, kind="ExternalInput")  # pre-transposed [d, e]
    wk = nc.dram_tensor("wk", [D, D], BF16, kind="ExternalInput")
    wv = nc.dram_tensor("wv", [D, D], BF16, kind="ExternalInput")
    # consts[p, 0:8]=bq  [p, 8:16]=bk  [p, 16:1040]=bv broadcast
    consts = nc.dram_tensor("consts", [P, 16 + D], F32, kind="ExternalInput")
    out = nc.dram_tensor("out", [T, D], F32, kind="ExternalOutput")

    with tile.TileContext(nc) as tc, ExitStack() as ctx:
        const = ctx.enter_context(tc.tile_pool(name="const", bufs=1))
        qkv = ctx.enter_context(tc.tile_pool(name="qkv", bufs=1))

        eps_t = const.tile([P, 1], F32)
        nc.vector.memset(eps_t, EPS)
        ones_t = const.tile([P, 1], BF16)
        nc.vector.memset(ones_t, 1.0)
        c_sb = const.tile([P, 16 + D], F32)
        bq_sb = c_sb[:, 0:N_EB]
        bk_sb = c_sb[:, N_EB:2 * N_EB]
        bv_bc = c_sb[:, 16:16 + D]

        # ---- persistent projection outputs ----
        # qT/kT in fp8e4 (e4m3): feeds the DoubleRow scores matmul at 2x rate.
        qT = qkv.tile([P, N_EB, T], F8E4)        # qT[p, eb, i] = q[i, eb*128+p]
        kT = qkv.tile([P, N_EB, T], F8E4)
        v_sb = qkv.tile([P, N_IT, D], BF16)      # v[p, jt, e] = v[jt*128+p, e]

        with tc.tile_pool(name="proj_phase", bufs=1) as pp, \
             tc.tile_pool(name="proj_ps", bufs=8, space="PSUM") as proj_ps:
            w_sbs = {}
            tp_hist = []  # last xbar-transpose instruction of each batch

            def order_after_transposes(dma_inst, lag=2):
                # Schedule copies after a past transpose batch: the scheduler
                # otherwise slots them inside a batch, and every transpose
                # serializes against in-flight copies (xbar guard). lag=2 lets
                # loads land in the gap between batches a window early; lag=1
                # keeps bulky weight loads strictly behind the current batch.
                tgt = tp_hist[-lag] if len(tp_hist) >= lag else \
                    (tp_hist[-1] if tp_hist else None)
                if tgt is not None:
                    add_dep_helper(dma_inst.ins, tgt.ins, False,
                                   reason="copy after transpose batch (xbar)")

            def load_weight(name, w_dram):
                # w_sb[p, a, e] = w[a*128+p, e]; 2KB contiguous descriptor rows
                w_sbs[name] = pp.tile([P, N_DB, D], BF16, tag=f"w_{name}", bufs=1,
                                      name=f"w_sb_{name}")
                wi = nc.gpsimd.dma_start(out=w_sbs[name],
                                         in_=w_dram[:, :].rearrange("(a p) e -> p a e", p=P))
                order_after_transposes(wi, lag=1)

            x_chunks = {}

            def load_chunk(pname, x_dram, ic):
                xc = pp.tile([P, CT, D], BF16, tag="x_chunk", bufs=5,
                             name=f"x_{pname}_{ic}")
                li = nc.gpsimd.dma_start(
                    out=xc,
                    in_=x_dram[ts(ic, 512), :].rearrange("(s p) d -> p s d", p=P),
                )
                order_after_transposes(li, lag=1)
                x_chunks[(pname, ic)] = xc

            def ln_chunk(pname, ic):
                """In-place layernorm of one loaded chunk; returns the tile."""
                xc = x_chunks.pop((pname, ic))
                held = []
                for s in range(CT):
                    stats = pp.tile([P, 2, 6], F32, tag="stats", bufs=8,
                                    name=f"stats_{pname}_{ic}_{s}")
                    for sb in range(2):
                        nc.vector.bn_stats(out=stats[:, sb, :], in_=xc[:, s, ts(sb, 512)])
                    mv = pp.tile([P, 2], F32, tag="mv", bufs=8,
                                 name=f"mv_{pname}_{ic}_{s}")
                    nc.vector.bn_aggr(out=mv, in_=stats)
                    rstd = pp.tile([P, 1], F32, tag="rstd", bufs=8,
                                   name=f"rstd_{pname}_{ic}_{s}")
                    nc.scalar.activation(
                        out=rstd, in_=mv[:, 1:2],
                        func=mybir.ActivationFunctionType.Abs_reciprocal_sqrt,
                        bias=eps_t,
                    )
                    held.append((s, mv, rstd))
                for s, mv, rstd in held:
                    nc.vector.tensor_scalar(
                        out=xc[:, s, :], in0=xc[:, s, :], scalar1=mv[:, 0:1],
                        scalar2=rstd,
                        op0=mybir.AluOpType.subtract, op1=mybir.AluOpType.mult,
                    )
                return xc

            def transpose_half(xcs, xt_half):
                """8 back-to-back xbar transposes: one batch, few mode flips.

                xt_half[p, db, u*128+q] = ln[u*128+q (token), db*128+p].
                """
                last = None
                for u, (xc, s) in enumerate(xcs):
                    last = nc.sync.dma_start(
                        out=xt_half[:, :, ts(u, P)], in_=xc[:, s, :], transpose=True)
                tp_hist.append(last)

            def proj_chunkT(w_sb, bias_sb, xt_half, off, dstT, pname, ic):
                """dstT[:, eb, ic*512:+512] = w^T @ ln_chunk + bias."""
                for eb in range(N_EB):
                    ps = proj_ps.tile([P, 512], F32, tag="mm", name=f"ps_{pname}_{eb}_{ic}")
                    for db in range(N_DB):
                        nc.tensor.matmul(
                            ps, lhsT=w_sb[:, db, ts(eb, P)],
                            rhs=xt_half[:, db, ds(off, 512)],
                            start=(db == 0), stop=(db == N_DB - 1),
                        )
                    nc.scalar.activation(
                        out=dstT[:, eb, ts(ic, 512)], in_=ps,
                        func=mybir.ActivationFunctionType.Identity,
                        bias=bias_sb[:, eb:eb + 1],
                    )

            def proj_chunkV(w_sb, xt_half, off, ic):
                """v_sb[:, ic*4+s, :] = ln_chunk^T @ w + bias (natural layout)."""
                for s in range(CT):
                    jt = ic * CT + s
                    for ec in range(N_EC):
                        ps = proj_ps.tile([P, 512], F32, tag="mm",
                                        name=f"ps_v_{jt}_{ec}")
                        for db in range(N_DB):
                            nc.tensor.matmul(
                                ps,
                                lhsT=xt_half[:, db, ds(off + s * P, P)],
                                rhs=w_sb[:, db, ts(ec, 512)],
                                start=(db == 0), stop=(db == N_DB - 1),
                            )
                        nc.vector.tensor_add(
                            out=v_sb[:, jt, ts(ec, 512)], in0=ps,
                            in1=bv_bc[:, ts(ec, 512)],
                        )

            # ---- schedule ----------------------------------------------------
            # Per phase: batches of chunks; each batch = loads+LN, then its
            # transposes back-to-back; proj of batch b-1 issued after the
            # transposes of batch b (one-batch software pipeline, PE stays fed).
            phases = [
                ("t", x_t, lambda xt, off, ic: proj_chunkT(w_sbs["q"], bq_sb, xt, off, qT, "q", ic)),
                ("k", x_k, lambda xt, off, ic: proj_chunkT(w_sbs["k"], bk_sb, xt, off, kT, "k", ic)),
                ("v", x_v, lambda xt, off, ic: proj_chunkV(w_sbs["v"], xt, off, ic)),
            ]
            # batches: phase t starts per-chunk to shorten the pipeline head
            batch_of = {"t": [[0], [1], [2, 3]], "k": [[0, 1], [2, 3]],
                        "v": [[0, 1], [2, 3]]}

            # global load order, prefetched a batch ahead
            order = [(pn, xd, ic) for pn, xd, _ in phases for ic in range(N_IC)]
            n_loaded = [0]

            def ensure_loaded(upto):
                while n_loaded[0] <= min(upto, len(order) - 1):
                    pn, xd, ic = order[n_loaded[0]]
                    load_chunk(pn, xd, ic)
                    n_loaded[0] += 1

            ensure_loaded(0)
            load_weight("q", wq)
            ensure_loaded(1)
            nc.gpsimd.dma_start(out=c_sb, in_=consts[:, :])
            # wk/wv issued after the first transpose batches: in-flight copies
            # block transposes (xbar guard), so keep the head's DMA queue lean
            post_batch = {("t", 0): lambda: load_weight("k", wk),
                          ("t", 1): lambda: load_weight("v", wv)}

            g = 0          # chunks fully issued through LN
            pending = None  # (proj_fn, xt_half, [(off, ic), ...]) awaiting proj
            for pname, x_dram, proj_fn in phases:
                with nc.named_scope(f"phase_{pname}"):
                    for bi, batch in enumerate(batch_of[pname]):
                        # proj of the previous batch first: its PSUM-evacuating
                        # ACTIVATEs must precede this batch's rstds on the
                        # Scalar queue (strict FIFO) or the PE stalls on PSUM
                        if pending is not None:
                            pfn, pxt, pchunks = pending
                            for off, ic in pchunks:
                                pfn(pxt, off, ic)
                        xcs = []
                        for ic in batch:
                            xc = ln_chunk(pname, ic)
                            xcs.extend((xc, s) for s in range(CT))
                            g += 1
                        xt_half = pp.tile([P, N_DB, len(batch) * 512], BF16,
                                          tag="xt", bufs=2,
                                          name=f"xt_{pname}_{batch[0]}")
                        transpose_half(xcs, xt_half)
                        # prefetch loads only BETWEEN transpose batches: a load
                        # scheduled among the transposes splits the batch (each
                        # transpose xbar-serializes against prior copies)
                        ensure_loaded(g + 2)
                        if (pname, bi) in post_batch:
                            post_batch[(pname, bi)]()
                        pending = (proj_fn, xt_half,
                                   [(j * 512, ic) for j, ic in enumerate(batch)])
            pfn, pxt, pchunks = pending
            for off, ic in pchunks:
                pfn(pxt, off, ic)

        # ---- attention, software-pipelined one chunk deep ----
        with tc.tile_pool(name="attv_ps", bufs=2, space="PSUM") as attv_ps, \
             tc.tile_pool(name="mm_ps", bufs=4, space="PSUM") as mm_ps, \
             tc.tile_pool(name="att", bufs=1) as att:
            aTs = {}

            def scores_chunk(ic):
                aT = att.tile([P, N_IT, 512], BF16, tag="aT", bufs=2, name=f"aT_{ic}")
                aTs[ic] = aT
                with nc.named_scope(f"scores_{ic}"):
                    for jt in range(N_IT):
                        ps = mm_ps.tile([P, 512], F32, tag="mm", name=f"ps_s_{ic}_{jt}")
                        # fp8e4 DoubleRow: contraction over eb pairs at 2x rate
                        for a in range(N_EB // 2):
                            nc.tensor.matmul(
                                ps, lhsT=kT[:, ds(2 * a, 2), ts(jt, P)],
                                rhs=qT[:, ds(2 * a, 2), ts(ic, 512)],
                                start=(a == 0), stop=(a == N_EB // 2 - 1),
                                perf_mode=mybir.MatmulPerfMode.DoubleRow,
                            )
                        nc.scalar.activation(
                            out=aT[:, jt, :], in_=ps,
                            func=mybir.ActivationFunctionType.Exp, scale=SCALE,
                        )

            def attv_chunk(ic):
                aT = aTs.pop(ic)
                with nc.named_scope(f"attv_{ic}"):
                    for isub in range(4):
                        ou = attv_ps.tile([P, D], F32, tag="ou", name=f"ou_{ic}_{isub}")
                        zz = mm_ps.tile([P, 1], F32, tag="mm", name=f"z_{ic}_{isub}")
                        # same-bank runs of 16 accumulating matmuls (bank cycling
                        # between consecutive matmuls forces PE micro-stalls)
                        for ec in range(N_EC):
                            for jt in range(N_IT):
                                nc.tensor.matmul(
                                    ou[:, ts(ec, 512)], lhsT=aT[:, jt, ts(isub, P)],
                                    rhs=v_sb[:, jt, ts(ec, 512)],
                                    start=(jt == 0), stop=(jt == N_IT - 1))
                        for jt in range(N_IT):
                            nc.tensor.matmul(zz, lhsT=aT[:, jt, ts(isub, P)], rhs=ones_t,
                                             start=(jt == 0), stop=(jt == N_IT - 1))
                        rz = att.tile([P, 1], F32, tag="rz", bufs=2,
                                      name=f"rz_{ic}_{isub}")
                        nc.vector.reciprocal(out=rz, in_=zz)
                        o_sb = att.tile([P, D], F32, tag="o_sb", bufs=2,
                                        name=f"o_{ic}_{isub}")
                        nc.vector.tensor_scalar_mul(out=o_sb, in0=ou, scalar1=rz)
                        nc.sync.dma_start(out=out[ts(ic * 4 + isub, P), :], in_=o_sb)

            scores_chunk(0)
            for ic in range(N_IC):
                if ic + 1 < N_IC:
                    scores_chunk(ic + 1)
                attv_chunk(ic)

    nc.compile()
    return nc


_NC_CACHE = None


def _get_module():
    global _NC_CACHE
    if _NC_CACHE is None:
        _NC_CACHE = build_module()
    return _NC_CACHE


def kernel(target, source_k, source_v, Wq, bq, Wk, bk, Wv, bv,
           g_t, b_t, g_k, b_k, g_v, b_v):
    target = np.asarray(target, dtype=np.float32)
    source_k = np.asarray(source_k, dtype=np.float32)
    source_v = np.asarray(source_v, dtype=np.float32)
    Wq = np.asarray(Wq, dtype=np.float32); bq = np.asarray(bq, dtype=np.float32)
    Wk = np.asarray(Wk, dtype=np.float32); bk = np.asarray(bk, dtype=np.float32)
    Wv = np.asarray(Wv, dtype=np.float32); bv = np.asarray(bv, dtype=np.float32)
    g_t = np.asarray(g_t, dtype=np.float32); b_t = np.asarray(b_t, dtype=np.float32)
    g_k = np.asarray(g_k, dtype=np.float32); b_k = np.asarray(b_k, dtype=np.float32)
    g_v = np.asarray(g_v, dtype=np.float32); b_v = np.asarray(b_v, dtype=np.float32)

    bf16 = ml_dtypes.bfloat16
    # Fold the layernorm affine (g, b) into the projection weights/biases:
    #   LN_affine(x) @ W.T + b  ==  LN_plain(x) @ (W*g).T + (b + W @ b_ln)
    wqT = np.ascontiguousarray((Wq * g_t[None, :]).T).astype(bf16)
    wkT = np.ascontiguousarray((Wk * g_k[None, :]).T).astype(bf16)
    wvT = np.ascontiguousarray((Wv * g_v[None, :]).T).astype(bf16)
    bq_f = bq + Wq @ b_t
    bk_f = bk + Wk @ b_k
    bv_f = bv + Wv @ b_v

    # consts[p, 0:8]=bq[a*128+p], [p, 8:16]=bk[a*128+p], [p, 16:]=bv broadcast
    consts = np.empty((P, 16 + D), np.float32)
    consts[:, 0:8] = bq_f.reshape(8, P).T
    consts[:, 8:16] = bk_f.reshape(8, P).T
    consts[:, 16:] = bv_f[None, :]

    x_t8 = target.astype(bf16)
    x_k8 = source_k.astype(bf16)
    x_v8 = source_v.astype(bf16)

    nc = _get_module()
    in_maps = []
    for b in range(B):
        in_maps.append({
            "x_t": np.ascontiguousarray(x_t8[b]),
            "x_k": np.ascontiguousarray(x_k8[b]),
            "x_v": np.ascontiguousarray(x_v8[b]),
            "wq": wqT, "wk": wkT, "wv": wvT,
            "consts": consts,
        })

    res = run_bass_kernel_spmd(nc, in_maps, core_ids=list(range(B)),
                               trace=bool(int(os.environ.get("KERNEL_TRACE", "0"))))
    out = np.stack([res.results[b]["out"] for b in range(B)], axis=0)
    kernel.last_results = res
    return out
